# revision 2
# baseline (speedup 1.0000x reference)
"""HBV hydrological model kernel for Trainium2 (Bass/Tile), 8-core basin-parallel.

Layout (per core):
  - 512 basins = 128 partitions x G=4 groups; basin b = g*128 + p.
  - DRAM arrays are [128, G*T] group-major per partition (col = g*T + t).
  - Sequential loop over T steps on [128, G] tiles (strided slices, step T).
"""

import numpy as np

import concourse.bacc as bacc
import concourse.bass as bass
import concourse.mybir as mybir
import concourse.tile as tile
from concourse import bass_utils

F32 = mybir.dt.float32
OP = mybir.AluOpType
AF = mybir.ActivationFunctionType

N_T = 2000
N_B = 4096
N_CORES = 8
BPC = N_B // N_CORES  # 512
G = BPC // 128  # 4

# Parameter bounds: TT,CFMAX,SFCF,CFR,CWH,FC,LP,BETA,PERC,UZL,K0,K1,K2,MAXBAS
P_MINS = np.array([-2.5, 0.5, 0.4, 0.0, 0.0, 50.0, 0.3, 1.0, 0.0, 0.0, 0.05, 0.01, 0.001, 1.0], np.float32)
P_MAXS = np.array([2.5, 10.0, 1.5, 0.1, 0.2, 700.0, 1.0, 6.0, 8.0, 100.0, 0.5, 0.3, 0.15, 7.0], np.float32)
iTT, iCFMAX, iSFCF, iCFR, iCWH, iFC, iLP, iBETA, iPERC, iUZL, iK0, iK1, iK2, iMAXBAS = range(14)


def build_kernel(T=N_T, TC=500):
    """Build the single-core SPMD bass program (same NEFF for all 8 cores)."""
    nc = bacc.Bacc("TRN2", target_bir_lowering=False)

    xP = nc.dram_tensor("xP", [128, G * T], F32, kind="ExternalInput")
    xT = nc.dram_tensor("xT", [128, G * T], F32, kind="ExternalInput")
    xE = nc.dram_tensor("xE", [128, G * T], F32, kind="ExternalInput")
    pars = nc.dram_tensor("pars", [128, 14 * G], F32, kind="ExternalInput")
    qout = nc.dram_tensor("q", [128, G * T], F32, kind="ExternalOutput")

    xP3 = xP[:].rearrange("p (g t) -> p g t", g=G)
    xT3 = xT[:].rearrange("p (g t) -> p g t", g=G)
    xE3 = xE[:].rearrange("p (g t) -> p g t", g=G)
    q3 = qout[:].rearrange("p (g t) -> p g t", g=G)

    n_chunks = T // TC
    assert T % TC == 0

    with tile.TileContext(nc) as tc:
        with (
            tc.tile_pool(name="const", bufs=1) as cpool,
            tc.tile_pool(name="inp", bufs=2) as ipool,
            tc.tile_pool(name="stream", bufs=2) as spool,
            tc.tile_pool(name="qp", bufs=2) as qpool,
            tc.tile_pool(name="state", bufs=3) as stpool,
            tc.tile_pool(name="tmp", bufs=3) as tpool,
        ):
            dve = nc.vector
            pool = nc.gpsimd
            act = nc.scalar

            # ---- parameter transform (one-time) ----
            praw = cpool.tile([128, 14 * G], F32, tag="praw", name="praw")
            nc.sync.dma_start(out=praw[:], in_=pars[:])
            psig = cpool.tile([128, 14 * G], F32, tag="psig", name="psig")
            act.activation(psig[:], praw[:], AF.Sigmoid)
            cpar = cpool.tile([128, 14 * G], F32, tag="cpar", name="cpar")
            for i in range(14):
                lo, hi = float(P_MINS[i]), float(P_MAXS[i])
                sl = slice(i * G, (i + 1) * G)
                dve.tensor_scalar(cpar[:, sl], psig[:, sl], hi - lo, lo, OP.mult, OP.add)

            def par(i):
                return cpar[:, i * G:(i + 1) * G]

            # derived parameter tiles
            TT5 = cpool.tile([128, G], F32, tag="TT5", name="TT5")
            dve.tensor_scalar_mul(TT5[:], par(iTT), 5.0)
            mTT = cpool.tile([128, G], F32, tag="mTT", name="mTT")
            dve.tensor_scalar_mul(mTT[:], par(iTT), -1.0)
            CFRCFM = cpool.tile([128, G], F32, tag="CFRCFM", name="CFRCFM")
            dve.tensor_mul(CFRCFM[:], par(iCFR), par(iCFMAX))
            invFC = cpool.tile([128, G], F32, tag="invFC", name="invFC")
            dve.reciprocal(invFC[:], par(iFC))
            invLP = cpool.tile([128, G], F32, tag="invLP", name="invLP")
            dve.reciprocal(invLP[:], par(iLP))
            c1t = cpool.tile([128, G], F32, tag="c1t", name="c1t")
            dve.tensor_mul(c1t[:], invFC[:], invLP[:])
            # FC^-BETA = exp(-BETA * ln FC)
            lnFC = cpool.tile([128, G], F32, tag="lnFC", name="lnFC")
            act.activation(lnFC[:], par(iFC), AF.Ln)
            mBlnFC = cpool.tile([128, G], F32, tag="mBlnFC", name="mBlnFC")
            dve.tensor_mul(mBlnFC[:], lnFC[:], par(iBETA))
            dve.tensor_scalar_mul(mBlnFC[:], mBlnFC[:], -1.0)
            invFCB = cpool.tile([128, G], F32, tag="invFCB", name="invFCB")
            act.activation(invFCB[:], mBlnFC[:], AF.Exp)

            # ---- initial state ----
            sp = stpool.tile([128, G], F32, tag="sp", name="sp")
            lw = stpool.tile([128, G], F32, tag="lw", name="lw")
            sm = stpool.tile([128, G], F32, tag="sm", name="sm")
            uz = stpool.tile([128, G], F32, tag="uz", name="uz")
            lz = stpool.tile([128, G], F32, tag="lz", name="lz")
            dve.memset(sp[:], 0.0)
            dve.memset(lw[:], 0.0)
            dve.memset(uz[:], 0.0)
            dve.memset(lz[:], 0.0)
            dve.tensor_scalar_mul(sm[:], par(iFC), 0.5)

            for ci in range(n_chunks):
                t0 = ci * TC
                # ---- load inputs for chunk ----
                xPc = ipool.tile([128, G * TC], F32, tag="xPc", name="xPc")
                xTc = ipool.tile([128, G * TC], F32, tag="xTc", name="xTc")
                xEc = ipool.tile([128, G * TC], F32, tag="xEc", name="xEc")
                nc.sync.dma_start(out=xPc[:].rearrange("p (g t) -> p g t", g=G), in_=xP3[:, :, t0:t0 + TC])
                nc.sync.dma_start(out=xTc[:].rearrange("p (g t) -> p g t", g=G), in_=xT3[:, :, t0:t0 + TC])
                nc.sync.dma_start(out=xEc[:].rearrange("p (g t) -> p g t", g=G), in_=xE3[:, :, t0:t0 + TC])

                # ---- batched precompute of forcing streams ----
                rain = spool.tile([128, G * TC], F32, tag="rain", name="rain")
                snw = spool.tile([128, G * TC], F32, tag="snw", name="snw")
                mcs = spool.tile([128, G * TC], F32, tag="mcs", name="mcs")
                rcs = spool.tile([128, G * TC], F32, tag="rcs", name="rcs")
                sfc = spool.tile([128, G * TC], F32, tag="sfc", name="sfc")
                for g in range(G):
                    gs = slice(g * TC, (g + 1) * TC)
                    Tg = xTc[:, gs]
                    Pg = xPc[:, gs]
                    # snow_frac = sigmoid((TT - T) * 5)
                    act.activation(sfc[:, gs], Tg, AF.Sigmoid, bias=TT5[:, g:g + 1], scale=-5.0)
                    # tmp = P * sf ; rain = P - tmp ; snow = tmp * SFCF
                    pool.tensor_mul(sfc[:, gs], Pg, sfc[:, gs])
                    pool.tensor_sub(rain[:, gs], Pg, sfc[:, gs])
                    pool.tensor_scalar_mul(snw[:, gs], sfc[:, gs], par(iSFCF)[:, g:g + 1])
                    # meltcap = CFMAX * relu(T - TT); rfcap = CFR*CFMAX * relu(TT - T)
                    act.activation(mcs[:, gs], Tg, AF.Relu, bias=mTT[:, g:g + 1], scale=1.0)
                    pool.tensor_scalar_mul(mcs[:, gs], mcs[:, gs], par(iCFMAX)[:, g:g + 1])
                    act.activation(rcs[:, gs], Tg, AF.Relu, bias=par(iTT)[:, g:g + 1], scale=-1.0)
                    pool.tensor_scalar_mul(rcs[:, gs], rcs[:, gs], CFRCFM[:, g:g + 1])

                rain3 = rain[:].rearrange("p (g t) -> p g t", g=G)
                snw3 = snw[:].rearrange("p (g t) -> p g t", g=G)
                mcs3 = mcs[:].rearrange("p (g t) -> p g t", g=G)
                rcs3 = rcs[:].rearrange("p (g t) -> p g t", g=G)
                xEc3 = xEc[:].rearrange("p (g t) -> p g t", g=G)

                qc = qpool.tile([128, G * TC], F32, tag="qc", name="qc")
                qc3 = qc[:].rearrange("p (g t) -> p g t", g=G)

                # ---- sequential time loop ----
                for tt in range(TC):
                    rn = rain3[:, :, tt]
                    sn = snw3[:, :, tt]
                    mc = mcs3[:, :, tt]
                    rc = rcs3[:, :, tt]
                    Ev = xEc3[:, :, tt]

                    def tp(tag):
                        return tpool.tile([128, G], F32, tag=tag, name=tag)

                    # --- snow routine ---
                    melt = tp("melt"); rf = tp("rf")
                    dve.tensor_tensor(melt[:], mc, sp[:], OP.min)
                    dve.tensor_tensor(rf[:], rc, lw[:], OP.min)
                    s1 = tp("s1"); s2 = tp("s2")
                    pool.tensor_add(s1[:], sp[:], sn)
                    dve.tensor_add(s2[:], s1[:], rf[:])
                    sp_n = stpool.tile([128, G], F32, tag="sp", name="sp")
                    dve.tensor_sub(sp_n[:], s2[:], melt[:])
                    l1 = tp("l1"); lw1 = tp("lw1")
                    pool.tensor_add(l1[:], lw[:], melt[:])
                    pool.tensor_sub(lw1[:], l1[:], rf[:])
                    cw = tp("cw"); ee = tp("ee"); rel = tp("rel")
                    dve.tensor_mul(cw[:], par(iCWH), sp_n[:])
                    dve.tensor_sub(ee[:], lw1[:], cw[:])
                    pool.tensor_scalar_max(rel[:], ee[:], 0.0)
                    lw_n = stpool.tile([128, G], F32, tag="lw", name="lw")
                    pool.tensor_sub(lw_n[:], lw1[:], rel[:])
                    wi = tp("wi")
                    dve.tensor_add(wi[:], rn, rel[:])

                    # --- soil routine ---
                    u1 = tp("u1"); u2 = tp("u2"); u3 = tp("u3"); aet = tp("aet")
                    pool.tensor_mul(u1[:], sm[:], c1t[:])
                    pool.tensor_scalar_min(u2[:], u1[:], 1.0)
                    pool.tensor_mul(u3[:], Ev, u2[:])
                    dve.tensor_tensor(aet[:], u3[:], sm[:], OP.min)
                    lg = tp("lg"); pb = tp("pb"); pw = tp("pw")
                    act.activation(lg[:], sm[:], AF.Ln)
                    dve.tensor_mul(pb[:], lg[:], par(iBETA))
                    act.activation(pw[:], pb[:], AF.Exp)
                    wipre = tp("wipre"); r0 = tp("r0"); rech = tp("rech")
                    pool.tensor_mul(wipre[:], wi[:], invFCB[:])
                    dve.tensor_mul(r0[:], wipre[:], pw[:])
                    dve.tensor_tensor(rech[:], r0[:], wi[:], OP.min)
                    q1s = tp("q1s"); q2s = tp("q2s"); q3s = tp("q3s")
                    dve.tensor_add(q1s[:], sm[:], wi[:])
                    dve.tensor_sub(q2s[:], q1s[:], aet[:])
                    dve.tensor_sub(q3s[:], q2s[:], rech[:])
                    sm_n = stpool.tile([128, G], F32, tag="sm", name="sm")
                    dve.scalar_tensor_tensor(sm_n[:], q3s[:], 0.0, par(iFC), OP.max, OP.min)

                    # --- response routine ---
                    perc = tp("perc"); e1 = tp("e1"); fast = tp("fast"); slow = tp("slow"); base = tp("base")
                    dve.tensor_tensor(perc[:], par(iPERC), uz[:], OP.min)
                    pool.tensor_sub(e1[:], uz[:], par(iUZL))
                    dve.scalar_tensor_tensor(fast[:], e1[:], 0.0, par(iK0), OP.max, OP.mult)
                    pool.tensor_mul(slow[:], par(iK1), uz[:])
                    pool.tensor_mul(base[:], par(iK2), lz[:])
                    qq1 = tp("qq1")
                    pool.tensor_add(qq1[:], fast[:], slow[:])
                    dve.tensor_add(qc3[:, :, tt], qq1[:], base[:])
                    v1 = tp("v1"); v2 = tp("v2"); v3 = tp("v3"); v4 = tp("v4")
                    dve.tensor_add(v1[:], uz[:], rech[:])
                    dve.tensor_sub(v2[:], v1[:], perc[:])
                    dve.tensor_sub(v3[:], v2[:], fast[:])
                    dve.tensor_sub(v4[:], v3[:], slow[:])
                    uz_n = stpool.tile([128, G], F32, tag="uz", name="uz")
                    dve.tensor_scalar_max(uz_n[:], v4[:], 0.0)
                    w1 = tp("w1"); w2 = tp("w2")
                    pool.tensor_add(w1[:], lz[:], perc[:])
                    pool.tensor_sub(w2[:], w1[:], base[:])
                    lz_n = stpool.tile([128, G], F32, tag="lz", name="lz")
                    pool.tensor_scalar_max(lz_n[:], w2[:], 0.0)

                    sp, lw, sm, uz, lz = sp_n, lw_n, sm_n, uz_n, lz_n

                # ---- store q chunk ----
                nc.sync.dma_start(out=q3[:, :, t0:t0 + TC], in_=qc3[:, :, :])

    nc.compile()
    return nc


def _prep_core_inputs(x_phy, parameters, core):
    b0 = core * BPC
    xs = x_phy[:, b0:b0 + BPC, :]  # [T, 512, 3]
    T = xs.shape[0]

    def comp(c):
        a = xs[:, :, c].reshape(T, G, 128)  # b = g*128 + p
        return np.ascontiguousarray(a.transpose(2, 1, 0).reshape(128, G * T))

    ps = parameters[b0:b0 + BPC, :].reshape(G, 128, 14)
    pp = np.ascontiguousarray(ps.transpose(1, 2, 0).reshape(128, 14 * G))
    return {"xP": comp(0), "xT": comp(1), "xE": comp(2), "pars": pp}


LAST_RESULT = None


def kernel(x_phy, parameters, _T=None, _trace=False):
    global LAST_RESULT
    x_phy = np.asarray(x_phy, dtype=np.float32)
    parameters = np.asarray(parameters, dtype=np.float32)
    T = _T or x_phy.shape[0]
    TC = 500 if T % 500 == 0 else max(d for d in range(1, T + 1) if T % d == 0 and d <= 500)

    nc = build_kernel(T=T, TC=TC)
    in_maps = [_prep_core_inputs(x_phy, parameters, c) for c in range(N_CORES)]
    res = bass_utils.run_bass_kernel_spmd(nc, in_maps, core_ids=list(range(N_CORES)), trace=_trace)
    LAST_RESULT = res

    out = np.empty((T, N_B), np.float32)
    for c in range(N_CORES):
        qc = res.results[c]["q"].reshape(128, G, T)  # [p, g, t]
        out[:, c * BPC:(c + 1) * BPC] = qc.transpose(2, 1, 0).reshape(T, BPC)
    return out[..., None]



# revision 3
# speedup vs baseline: 1.7645x; 1.7645x over previous
"""HBV hydrological model kernel for Trainium2 (Bass/Tile), 8-core basin-parallel.

Layout (per core):
  - 512 basins = 128 partitions x G=4 groups; basin b = g*128 + p.
  - DRAM arrays are [128, G*T] group-major per partition (col = g*T + t).
  - Sequential loop over T steps on [128, G] tiles (strided slices, step T).
"""

import numpy as np

import concourse.bacc as bacc
import concourse.bass as bass
import concourse.mybir as mybir
import concourse.tile as tile
from concourse import bass_utils

F32 = mybir.dt.float32
OP = mybir.AluOpType
AF = mybir.ActivationFunctionType

N_T = 2000
N_B = 4096
N_CORES = 8
BPC = N_B // N_CORES  # 512
G = BPC // 128  # 4

# Parameter bounds: TT,CFMAX,SFCF,CFR,CWH,FC,LP,BETA,PERC,UZL,K0,K1,K2,MAXBAS
P_MINS = np.array([-2.5, 0.5, 0.4, 0.0, 0.0, 50.0, 0.3, 1.0, 0.0, 0.0, 0.05, 0.01, 0.001, 1.0], np.float32)
P_MAXS = np.array([2.5, 10.0, 1.5, 0.1, 0.2, 700.0, 1.0, 6.0, 8.0, 100.0, 0.5, 0.3, 0.15, 7.0], np.float32)
iTT, iCFMAX, iSFCF, iCFR, iCWH, iFC, iLP, iBETA, iPERC, iUZL, iK0, iK1, iK2, iMAXBAS = range(14)


def build_kernel(T=N_T, TC=500):
    """Build the single-core SPMD bass program (same NEFF for all 8 cores)."""
    nc = bacc.Bacc("TRN2", target_bir_lowering=False)

    xP = nc.dram_tensor("xP", [128, G * T], F32, kind="ExternalInput")
    xT = nc.dram_tensor("xT", [128, G * T], F32, kind="ExternalInput")
    xE = nc.dram_tensor("xE", [128, G * T], F32, kind="ExternalInput")
    pars = nc.dram_tensor("pars", [128, 14 * G], F32, kind="ExternalInput")
    qout = nc.dram_tensor("q", [128, G * T], F32, kind="ExternalOutput")

    xP3 = xP[:].rearrange("p (g t) -> p g t", g=G)
    xT3 = xT[:].rearrange("p (g t) -> p g t", g=G)
    xE3 = xE[:].rearrange("p (g t) -> p g t", g=G)
    q3 = qout[:].rearrange("p (g t) -> p g t", g=G)

    n_chunks = T // TC
    assert T % TC == 0

    with tile.TileContext(nc) as tc:
        with (
            tc.tile_pool(name="const", bufs=1) as cpool,
            tc.tile_pool(name="inp", bufs=2) as ipool,
            tc.tile_pool(name="stream", bufs=2) as spool,
            tc.tile_pool(name="qp", bufs=2) as qpool,
            tc.tile_pool(name="state", bufs=3) as stpool,
            tc.tile_pool(name="tmp", bufs=3) as tpool,
        ):
            dve = nc.vector
            pool = nc.gpsimd
            act = nc.scalar

            # ---- parameter transform (one-time) ----
            praw = cpool.tile([128, 14 * G], F32, tag="praw", name="praw")
            nc.sync.dma_start(out=praw[:], in_=pars[:])
            psig = cpool.tile([128, 14 * G], F32, tag="psig", name="psig")
            act.activation(psig[:], praw[:], AF.Sigmoid)
            cpar = cpool.tile([128, 14 * G], F32, tag="cpar", name="cpar")
            for i in range(14):
                lo, hi = float(P_MINS[i]), float(P_MAXS[i])
                sl = slice(i * G, (i + 1) * G)
                dve.tensor_scalar(cpar[:, sl], psig[:, sl], hi - lo, lo, OP.mult, OP.add)

            def par(i):
                return cpar[:, i * G:(i + 1) * G]

            # derived parameter tiles
            TT5 = cpool.tile([128, G], F32, tag="TT5", name="TT5")
            dve.tensor_scalar_mul(TT5[:], par(iTT), 5.0)
            mTT = cpool.tile([128, G], F32, tag="mTT", name="mTT")
            dve.tensor_scalar_mul(mTT[:], par(iTT), -1.0)
            CFRCFM = cpool.tile([128, G], F32, tag="CFRCFM", name="CFRCFM")
            dve.tensor_mul(CFRCFM[:], par(iCFR), par(iCFMAX))
            invFC = cpool.tile([128, G], F32, tag="invFC", name="invFC")
            dve.reciprocal(invFC[:], par(iFC))
            invLP = cpool.tile([128, G], F32, tag="invLP", name="invLP")
            dve.reciprocal(invLP[:], par(iLP))
            c1t = cpool.tile([128, G], F32, tag="c1t", name="c1t")
            dve.tensor_mul(c1t[:], invFC[:], invLP[:])
            # FC^-BETA = exp(-BETA * ln FC)
            lnFC = cpool.tile([128, G], F32, tag="lnFC", name="lnFC")
            act.activation(lnFC[:], par(iFC), AF.Ln)
            mBlnFC = cpool.tile([128, G], F32, tag="mBlnFC", name="mBlnFC")
            dve.tensor_mul(mBlnFC[:], lnFC[:], par(iBETA))
            dve.tensor_scalar_mul(mBlnFC[:], mBlnFC[:], -1.0)
            invFCB = cpool.tile([128, G], F32, tag="invFCB", name="invFCB")
            act.activation(invFCB[:], mBlnFC[:], AF.Exp)

            # ---- initial state ----
            sp = stpool.tile([128, G], F32, tag="sp", name="sp")
            lw = stpool.tile([128, G], F32, tag="lw", name="lw")
            sm = stpool.tile([128, G], F32, tag="sm", name="sm")
            uz = stpool.tile([128, G], F32, tag="uz", name="uz")
            lz = stpool.tile([128, G], F32, tag="lz", name="lz")
            dve.memset(sp[:], 0.0)
            dve.memset(lw[:], 0.0)
            dve.memset(uz[:], 0.0)
            dve.memset(lz[:], 0.0)
            dve.tensor_scalar_mul(sm[:], par(iFC), 0.5)

            for ci in range(n_chunks):
                t0 = ci * TC
                # ---- load inputs for chunk ----
                xPc = ipool.tile([128, G * TC], F32, tag="xPc", name="xPc")
                xTc = ipool.tile([128, G * TC], F32, tag="xTc", name="xTc")
                xEc = ipool.tile([128, G * TC], F32, tag="xEc", name="xEc")
                nc.sync.dma_start(out=xPc[:].rearrange("p (g t) -> p g t", g=G), in_=xP3[:, :, t0:t0 + TC])
                nc.sync.dma_start(out=xTc[:].rearrange("p (g t) -> p g t", g=G), in_=xT3[:, :, t0:t0 + TC])
                nc.sync.dma_start(out=xEc[:].rearrange("p (g t) -> p g t", g=G), in_=xE3[:, :, t0:t0 + TC])

                # ---- batched precompute of forcing streams ----
                rain = spool.tile([128, G * TC], F32, tag="rain", name="rain")
                snw = spool.tile([128, G * TC], F32, tag="snw", name="snw")
                mcs = spool.tile([128, G * TC], F32, tag="mcs", name="mcs")
                rcs = spool.tile([128, G * TC], F32, tag="rcs", name="rcs")
                sfc = spool.tile([128, G * TC], F32, tag="sfc", name="sfc")
                for g in range(G):
                    gs = slice(g * TC, (g + 1) * TC)
                    Tg = xTc[:, gs]
                    Pg = xPc[:, gs]
                    # snow_frac = sigmoid((TT - T) * 5)
                    act.activation(sfc[:, gs], Tg, AF.Sigmoid, bias=TT5[:, g:g + 1], scale=-5.0)
                    # tmp = P * sf ; rain = P - tmp ; snow = tmp * SFCF
                    pool.tensor_mul(sfc[:, gs], Pg, sfc[:, gs])
                    pool.tensor_sub(rain[:, gs], Pg, sfc[:, gs])
                    pool.tensor_scalar_mul(snw[:, gs], sfc[:, gs], par(iSFCF)[:, g:g + 1])
                    # meltcap = CFMAX * relu(T - TT); rfcap = CFR*CFMAX * relu(TT - T)
                    act.activation(mcs[:, gs], Tg, AF.Relu, bias=mTT[:, g:g + 1], scale=1.0)
                    pool.tensor_scalar_mul(mcs[:, gs], mcs[:, gs], par(iCFMAX)[:, g:g + 1])
                    act.activation(rcs[:, gs], Tg, AF.Relu, bias=par(iTT)[:, g:g + 1], scale=-1.0)
                    pool.tensor_scalar_mul(rcs[:, gs], rcs[:, gs], CFRCFM[:, g:g + 1])

                rain3 = rain[:].rearrange("p (g t) -> p g t", g=G)
                snw3 = snw[:].rearrange("p (g t) -> p g t", g=G)
                mcs3 = mcs[:].rearrange("p (g t) -> p g t", g=G)
                rcs3 = rcs[:].rearrange("p (g t) -> p g t", g=G)
                xEc3 = xEc[:].rearrange("p (g t) -> p g t", g=G)

                qc = qpool.tile([128, G * TC], F32, tag="qc", name="qc")
                qc3 = qc[:].rearrange("p (g t) -> p g t", g=G)

                # ---- sequential time loop ----
                for tt in range(TC):
                    rn = rain3[:, :, tt]
                    sn = snw3[:, :, tt]
                    mc = mcs3[:, :, tt]
                    rc = rcs3[:, :, tt]
                    Ev = xEc3[:, :, tt]

                    def tp(tag):
                        return tpool.tile([128, G], F32, tag=tag, name=tag)

                    # --- snow routine ---
                    melt = tp("melt"); rf = tp("rf")
                    dve.tensor_tensor(melt[:], mc, sp[:], OP.min)
                    dve.tensor_tensor(rf[:], rc, lw[:], OP.min)
                    s1 = tp("s1"); s2 = tp("s2")
                    pool.tensor_add(s1[:], sp[:], sn)
                    dve.tensor_add(s2[:], s1[:], rf[:])
                    sp_n = stpool.tile([128, G], F32, tag="sp", name="sp")
                    dve.tensor_sub(sp_n[:], s2[:], melt[:])
                    l1 = tp("l1"); lw1 = tp("lw1")
                    pool.tensor_add(l1[:], lw[:], melt[:])
                    pool.tensor_sub(lw1[:], l1[:], rf[:])
                    cw = tp("cw"); ee = tp("ee"); rel = tp("rel")
                    dve.tensor_mul(cw[:], par(iCWH), sp_n[:])
                    dve.tensor_sub(ee[:], lw1[:], cw[:])
                    pool.tensor_scalar_max(rel[:], ee[:], 0.0)
                    lw_n = stpool.tile([128, G], F32, tag="lw", name="lw")
                    pool.tensor_sub(lw_n[:], lw1[:], rel[:])
                    wi = tp("wi")
                    dve.tensor_add(wi[:], rn, rel[:])

                    # --- soil routine ---
                    u1 = tp("u1"); u2 = tp("u2"); u3 = tp("u3"); aet = tp("aet")
                    pool.tensor_mul(u1[:], sm[:], c1t[:])
                    pool.tensor_scalar_min(u2[:], u1[:], 1.0)
                    pool.tensor_mul(u3[:], Ev, u2[:])
                    dve.tensor_tensor(aet[:], u3[:], sm[:], OP.min)
                    lg = tp("lg"); pb = tp("pb"); pw = tp("pw")
                    act.activation(lg[:], sm[:], AF.Ln)
                    dve.tensor_mul(pb[:], lg[:], par(iBETA))
                    act.activation(pw[:], pb[:], AF.Exp)
                    wipre = tp("wipre"); r0 = tp("r0"); rech = tp("rech")
                    pool.tensor_mul(wipre[:], wi[:], invFCB[:])
                    dve.tensor_mul(r0[:], wipre[:], pw[:])
                    dve.tensor_tensor(rech[:], r0[:], wi[:], OP.min)
                    q1s = tp("q1s"); q2s = tp("q2s"); q3s = tp("q3s")
                    dve.tensor_add(q1s[:], sm[:], wi[:])
                    dve.tensor_sub(q2s[:], q1s[:], aet[:])
                    dve.tensor_sub(q3s[:], q2s[:], rech[:])
                    sm_n = stpool.tile([128, G], F32, tag="sm", name="sm")
                    dve.scalar_tensor_tensor(sm_n[:], q3s[:], 0.0, par(iFC), OP.max, OP.min)

                    # --- response routine ---
                    perc = tp("perc"); e1 = tp("e1"); fast = tp("fast"); slow = tp("slow"); base = tp("base")
                    dve.tensor_tensor(perc[:], par(iPERC), uz[:], OP.min)
                    pool.tensor_sub(e1[:], uz[:], par(iUZL))
                    dve.scalar_tensor_tensor(fast[:], e1[:], 0.0, par(iK0), OP.max, OP.mult)
                    pool.tensor_mul(slow[:], par(iK1), uz[:])
                    pool.tensor_mul(base[:], par(iK2), lz[:])
                    qq1 = tp("qq1")
                    pool.tensor_add(qq1[:], fast[:], slow[:])
                    dve.tensor_add(qc3[:, :, tt], qq1[:], base[:])
                    v1 = tp("v1"); v2 = tp("v2"); v3 = tp("v3"); v4 = tp("v4")
                    dve.tensor_add(v1[:], uz[:], rech[:])
                    dve.tensor_sub(v2[:], v1[:], perc[:])
                    dve.tensor_sub(v3[:], v2[:], fast[:])
                    dve.tensor_sub(v4[:], v3[:], slow[:])
                    uz_n = stpool.tile([128, G], F32, tag="uz", name="uz")
                    dve.tensor_scalar_max(uz_n[:], v4[:], 0.0)
                    w1 = tp("w1"); w2 = tp("w2")
                    pool.tensor_add(w1[:], lz[:], perc[:])
                    pool.tensor_sub(w2[:], w1[:], base[:])
                    lz_n = stpool.tile([128, G], F32, tag="lz", name="lz")
                    pool.tensor_scalar_max(lz_n[:], w2[:], 0.0)

                    sp, lw, sm, uz, lz = sp_n, lw_n, sm_n, uz_n, lz_n

                # ---- store q chunk ----
                nc.sync.dma_start(out=q3[:, :, t0:t0 + TC], in_=qc3[:, :, :])

    _compile_with_act_tables(nc, keep=("sigmoid_and_others", "natural_log_exp_and_others"))
    return nc


def _compile_with_act_tables(nc, keep):
    """Compile while restricting activation-table choice so ln/exp share one
    table (otherwise the compiler reloads a 1.3us act table before every Ln
    and Exp in the time loop)."""
    import concourse.bacc as bacc_mod

    orig = bacc_mod.get_activation_tables

    def patched(arch):
        tabs = orig(arch)
        return {k: (v if k in keep else set()) for k, v in tabs.items()}

    bacc_mod.get_activation_tables = patched
    try:
        nc.compile()
    finally:
        bacc_mod.get_activation_tables = orig


def _prep_core_inputs(x_phy, parameters, core):
    b0 = core * BPC
    xs = x_phy[:, b0:b0 + BPC, :]  # [T, 512, 3]
    T = xs.shape[0]

    def comp(c):
        a = xs[:, :, c].reshape(T, G, 128)  # b = g*128 + p
        return np.ascontiguousarray(a.transpose(2, 1, 0).reshape(128, G * T))

    ps = parameters[b0:b0 + BPC, :].reshape(G, 128, 14)
    pp = np.ascontiguousarray(ps.transpose(1, 2, 0).reshape(128, 14 * G))
    return {"xP": comp(0), "xT": comp(1), "xE": comp(2), "pars": pp}


LAST_RESULT = None


def kernel(x_phy, parameters, _T=None, _trace=False):
    global LAST_RESULT
    x_phy = np.asarray(x_phy, dtype=np.float32)
    parameters = np.asarray(parameters, dtype=np.float32)
    T = _T or x_phy.shape[0]
    TC = 500 if T % 500 == 0 else max(d for d in range(1, T + 1) if T % d == 0 and d <= 500)

    nc = build_kernel(T=T, TC=TC)
    in_maps = [_prep_core_inputs(x_phy, parameters, c) for c in range(N_CORES)]
    res = bass_utils.run_bass_kernel_spmd(nc, in_maps, core_ids=list(range(N_CORES)), trace=_trace)
    LAST_RESULT = res

    out = np.empty((T, N_B), np.float32)
    for c in range(N_CORES):
        qc = res.results[c]["q"].reshape(128, G, T)  # [p, g, t]
        out[:, c * BPC:(c + 1) * BPC] = qc.transpose(2, 1, 0).reshape(T, BPC)
    return out[..., None]

